# revision 5
# baseline (speedup 1.0000x reference)
"""Gemma sliding-window attention (S=2048, HID=3584, H=16, KV=8, D=256) on 8 trn2 cores.

Sharding: tensor-parallel by heads. Core c owns q heads {2c, 2c+1} and kv head c
(GQA stays local). Each core computes its partial o_proj output transposed
([HID, S]); the host sums the 8 partials (the "all-reduce") and transposes back.

Device-side dataflow per core (all matmuls fp32r = full PE rate):
  qkv   : out[tok, col] += hid_T[k, tok-tile].T @ w_qkv[k, col]   (hid_T stationary)
  norm  : fused DVE tensor_tensor_reduce -> eps + mean(x^2); sqrt; recip
  rope  : fused DVE scalar_tensor_tensor (x*rstd)*cos' + swap(x*rstd)*sin'
          with (1+norm_w) folded into host-precomputed cos'/sin'
  q/k^T : PE transpose (128x128 blocks) -> [D, S] layout for scores
  scores: s^T[t, s] = k^T.T @ q^T ; sliding-window mask added (-1e38) pre-tanh
  p     : exp(50*tanh(s*scaling/50) - 50)  -- fixed max=50 (softcap bound), no row max
  attnV : ao^T[d, s] += v[t, d].T @ p^T[t, s] ; denominator via ones-vector matmul
  norm2 : p sums reciprocal (approx-fast) broadcast over partitions, multiply
  o_proj: out_T[hid, s] += w_o[hd, hid-tile].T @ ao^T[hd, s] ; DMA psum -> dram
"""

import numpy as np

import concourse.bass as bass
import concourse.tile as tile
import concourse.mybir as mybir
from concourse import bacc
from concourse.bass_utils import run_bass_kernel_spmd

S, HID, H, KV, D = 2048, 3584, 16, 8, 256
WINDOW = 1024
SOFTCAP = 50.0
SCALING = 256 ** (-0.5)
EPS = 1e-6

N_CORES = 8
QH = H // N_CORES        # 2 q heads per core
MT = S // 128            # 16 token tiles
KT = HID // 128          # 28 hid tiles
SCW = 512                # s-chunk width
SC_N = S // SCW          # 4 s-chunks
NEG = -1e38

F32 = mybir.dt.float32
F32R = mybir.dt.float32r
AF = mybir.ActivationFunctionType
ALU = mybir.AluOpType

# additive-mask tile ids by r = 512*sc - 128*t
MASK_RS = [0, -128, -256, -384, 640, 768, 896, 1024]
MASK_IDX = {r: i for i, r in enumerate(MASK_RS)}


def _build_nc():
    nc = bacc.Bacc("TRN2", target_bir_lowering=False, debug=False,
                   num_devices=N_CORES)
    hid_t = nc.dram_tensor("hid_t", [HID, S], F32R, kind="ExternalInput")
    w_qkv = nc.dram_tensor("w_qkv", [HID, 4 * D], F32R, kind="ExternalInput")
    w_o = nc.dram_tensor("w_o", [QH * D, HID], F32R, kind="ExternalInput")
    cos_q = nc.dram_tensor("cos_q", [S, D], F32, kind="ExternalInput")
    sin_q = nc.dram_tensor("sin_q", [S, D], F32, kind="ExternalInput")
    cos_k = nc.dram_tensor("cos_k", [S, D], F32, kind="ExternalInput")
    sin_k = nc.dram_tensor("sin_k", [S, D], F32, kind="ExternalInput")
    masks_d = nc.dram_tensor("masks", [len(MASK_RS) * 128, SCW], F32,
                             kind="ExternalInput")
    ident_d = nc.dram_tensor("identity", [128, 128], F32, kind="ExternalInput")
    ones_d = nc.dram_tensor("ones", [128, 1], F32R, kind="ExternalInput")
    out_t = nc.dram_tensor("out_t", [HID, S], F32, kind="ExternalOutput")

    hv = hid_t.rearrange("(k p) s -> k p s", p=128)
    wv = w_qkv.rearrange("(k p) c -> k p c", p=128)
    wov = w_o.rearrange("(k p) m -> k p m", p=128)
    mv = masks_d.rearrange("(i p) f -> i p f", p=128)

    with tile.TileContext(nc) as tc:
        with tc.tile_pool(name="pers", bufs=1) as pers:
            qt_sb = pers.tile([128, 2 * QH, S], F32R)   # q^T  (h*2+d, tok)
            kt_sb = pers.tile([128, 2, S], F32R)        # k^T  (d, tok)
            v_sb = pers.tile([128, MT, D], F32R)        # v    (tok-tile, d)
            ident_sb = pers.tile([128, 128], F32)
            ones_sb = pers.tile([128, 1], F32R)
            nc.sync.dma_start(out=ident_sb[:], in_=ident_d[:, :])
            nc.sync.dma_start(out=ones_sb[:], in_=ones_d[:, :])

            # ---------------- phase 1: qkv + norm + rope + transpose -------
            with tc.tile_pool(name="wq", bufs=1) as wqp, \
                 tc.tile_pool(name="htp", bufs=6) as htp, \
                 tc.tile_pool(name="csp", bufs=2) as csp, \
                 tc.tile_pool(name="scr", bufs=2) as scr, \
                 tc.tile_pool(name="psqkv", bufs=2, space="PSUM") as psqkv, \
                 tc.tile_pool(name="pstr", bufs=2, space="PSUM") as pstr:
                wq_sb = wqp.tile([128, KT, 4 * D], F32R)
                for k in range(KT):
                    nc.sync.dma_start(out=wq_sb[:, k, :], in_=wv[k])
                eps_sb = wqp.tile([128, 1], F32)
                nc.vector.memset(eps_sb[:], EPS)

                for m in range(MT):
                    ms = slice(m * 128, (m + 1) * 128)
                    cq = csp.tile([128, D], F32, tag="cq")
                    sq = csp.tile([128, D], F32, tag="sq")
                    ck = csp.tile([128, D], F32, tag="ck")
                    sk = csp.tile([128, D], F32, tag="sk")
                    nc.sync.dma_start(out=cq[:], in_=cos_q[ms, :])
                    nc.sync.dma_start(out=sq[:], in_=sin_q[ms, :])
                    nc.sync.dma_start(out=ck[:], in_=cos_k[ms, :])
                    nc.sync.dma_start(out=sk[:], in_=sin_k[ms, :])

                    ps_q = psqkv.tile([128, 512], F32, tag="ps_q")
                    ps_kv = psqkv.tile([128, 512], F32, tag="ps_kv")
                    for k in range(KT):
                        ht = htp.tile([128, 128], F32R, tag="ht")
                        nc.sync.dma_start(out=ht[:], in_=hv[k, :, ms])
                        nc.tensor.matmul(ps_q[:], ht[:], wq_sb[:, k, 0:512],
                                         start=(k == 0), stop=(k == KT - 1))
                        nc.tensor.matmul(ps_kv[:], ht[:], wq_sb[:, k, 512:1024],
                                         start=(k == 0), stop=(k == KT - 1))

                    # v straight out of psum
                    nc.scalar.copy(v_sb[:, m, :], ps_kv[:, 256:512])

                    # per-head rms norm + rope + transpose
                    for h in range(3):  # 0,1 = q heads; 2 = k head
                        if h < 2:
                            src = ps_q[:, h * 256:(h + 1) * 256]
                            cosb, sinb = cq, sq
                        else:
                            src = ps_kv[:, 0:256]
                            cosb, sinb = ck, sk
                        sq_scr = scr.tile([128, D], F32, tag="sq_scr")
                        var = scr.tile([128, 1], F32, tag="var")
                        nc.scalar.activation(sq_scr[:], src, AF.Square,
                                             accum_out=var[:])
                        sd = scr.tile([128, 1], F32, tag="sd")
                        nc.scalar.activation(sd[:], var[:], AF.Sqrt,
                                             bias=eps_sb[:], scale=1.0 / D)
                        rstd = scr.tile([128, 1], F32, tag="rstd")
                        nc.vector.reciprocal(rstd[:], sd[:])

                        t1 = scr.tile([128, D], F32, tag="t1")
                        nc.vector.scalar_tensor_tensor(
                            out=t1[:], in0=src, scalar=rstd[:], in1=cosb[:],
                            op0=ALU.mult, op1=ALU.mult)
                        t2 = scr.tile([128, D], F32, tag="t2")
                        nc.vector.scalar_tensor_tensor(
                            out=t2[:, 0:128], in0=src[:, 128:256],
                            scalar=rstd[:], in1=sinb[:, 0:128],
                            op0=ALU.mult, op1=ALU.mult)
                        nc.vector.scalar_tensor_tensor(
                            out=t2[:, 128:256], in0=src[:, 0:128],
                            scalar=rstd[:], in1=sinb[:, 128:256],
                            op0=ALU.mult, op1=ALU.mult)
                        qr = scr.tile([128, D], F32, tag="qr")
                        nc.vector.tensor_add(qr[:], t1[:], t2[:])

                        for d in range(2):
                            ps_t = pstr.tile([128, 128], F32, tag="tr")
                            nc.tensor.transpose(ps_t[:], qr[:, d * 128:(d + 1) * 128],
                                                ident_sb[:])
                            if h < 2:
                                dst = qt_sb[:, h * 2 + d, ms]
                            else:
                                dst = kt_sb[:, d, ms]
                            nc.scalar.copy(dst, ps_t[:])

            # ---------------- phase 2+3: attention + o_proj ----------------
            with tc.tile_pool(name="ph2", bufs=1) as ph2, \
                 tc.tile_pool(name="t2p", bufs=2) as t2p, \
                 tc.tile_pool(name="psS", bufs=2, space="PSUM") as psS, \
                 tc.tile_pool(name="psO", bufs=1, space="PSUM") as psO, \
                 tc.tile_pool(name="psL", bufs=2, space="PSUM") as psL, \
                 tc.tile_pool(name="psM", bufs=2, space="PSUM") as psM:
                ao_sb = ph2.tile([128, 2 * QH, S], F32R)  # attn-out^T (h*2+d)
                mask_sb = ph2.tile([128, len(MASK_RS), SCW], F32)
                for i in range(len(MASK_RS)):
                    nc.sync.dma_start(out=mask_sb[:, i, :], in_=mv[i])
                wo_sb = ph2.tile([128, 2 * QH, HID], F32R)
                for kt in range(2 * QH):
                    nc.sync.dma_start(out=wo_sb[:, kt, :], in_=wov[kt])
                exp_bias = ph2.tile([128, 1], F32)
                nc.vector.memset(exp_bias[:], -SOFTCAP)

                for sc in range(SC_N):
                    ss = slice(sc * SCW, (sc + 1) * SCW)
                    t_lo = max(0, 4 * sc - 8)
                    t_hi = min(MT - 1, 4 * sc + 3)
                    for h in range(QH):
                        ps_o0 = psO.tile([128, SCW], F32, tag="o0")
                        ps_o1 = psO.tile([128, SCW], F32, tag="o1")
                        ps_od = (ps_o0, ps_o1)
                        ps_l = psL.tile([1, SCW], F32, tag="l")
                        n_t = t_hi - t_lo + 1
                        for ti, t in enumerate(range(t_lo, t_hi + 1)):
                            r = SCW * sc - 128 * t
                            ts_ = slice(t * 128, (t + 1) * 128)
                            ps_s = psS.tile([128, SCW], F32, tag="s")
                            for d in range(2):
                                nc.tensor.matmul(
                                    ps_s[:], kt_sb[:, d, ts_],
                                    qt_sb[:, h * 2 + d, ss],
                                    start=(d == 0), stop=(d == 1))
                            if r in MASK_IDX:
                                stm = t2p.tile([128, SCW], F32, tag="stm")
                                nc.vector.tensor_add(
                                    stm[:], ps_s[:], mask_sb[:, MASK_IDX[r], :])
                                tanh_src = stm[:]
                            else:
                                tanh_src = ps_s[:]
                            st = t2p.tile([128, SCW], F32, tag="st")
                            nc.scalar.activation(st[:], tanh_src, AF.Tanh,
                                                 scale=float(SCALING / SOFTCAP))
                            p = t2p.tile([128, SCW], F32R, tag="p")
                            nc.scalar.activation(p[:], st[:], AF.Exp,
                                                 bias=exp_bias[:], scale=SOFTCAP)
                            for d in range(2):
                                nc.tensor.matmul(
                                    ps_od[d][:],
                                    v_sb[:, t, d * 128:(d + 1) * 128], p[:],
                                    start=(ti == 0), stop=(ti == n_t - 1))
                            nc.tensor.matmul(ps_l[0:1, :], ones_sb[:], p[:],
                                             start=(ti == 0), stop=(ti == n_t - 1))
                        # softmax denominator -> reciprocal -> broadcast
                        l_row = t2p.tile([1, SCW], F32, tag="lrow")
                        nc.scalar.copy(l_row[0:1, :], ps_l[0:1, :])
                        rl = t2p.tile([1, SCW], F32, tag="rl")
                        nc.vector.reciprocal_approx_fast(rl[0:1, :], l_row[0:1, :])
                        rlb = t2p.tile([128, SCW], F32, tag="rlb")
                        nc.gpsimd.partition_broadcast(rlb[:], rl[0:1, :])
                        for d in range(2):
                            nc.vector.tensor_mul(ao_sb[:, h * 2 + d, ss],
                                                 ps_od[d][:], rlb[:])
                    # o_proj for this s-chunk
                    for M in range(KT):
                        ps_out = psM.tile([128, SCW], F32, tag="mm")
                        for kt in range(2 * QH):
                            nc.tensor.matmul(ps_out[:],
                                             wo_sb[:, kt, M * 128:(M + 1) * 128],
                                             ao_sb[:, kt, ss],
                                             start=(kt == 0), stop=(kt == 2 * QH - 1))
                        o_ev = t2p.tile([128, SCW], F32, tag="oev")
                        if M % 2 == 0:
                            nc.scalar.copy(o_ev[:], ps_out[:])
                        else:
                            nc.vector.tensor_copy(o_ev[:], ps_out[:])
                        nc.sync.dma_start(out=out_t[M * 128:(M + 1) * 128, ss],
                                          in_=o_ev[:])
    nc.finalize()
    return nc


_NC = None


def _get_nc():
    global _NC
    if _NC is None:
        _NC = _build_nc()
    return _NC


def _host_inputs(hidden_states, w_qkv, w_o, q_norm_w, k_norm_w, cos, sin):
    """Build per-core in_maps (host-side sharding + precompute)."""
    f32 = np.float32
    hid_t = np.ascontiguousarray(hidden_states.T.astype(f32))

    wq_eff = (1.0 + q_norm_w.astype(f32))
    wk_eff = (1.0 + k_norm_w.astype(f32))

    def rope_tabs(w_eff):
        c = (cos.astype(f32) * w_eff[None, :])
        s_ = np.empty_like(c)
        s_[:, :128] = -sin[:, :128].astype(f32) * w_eff[None, 128:]
        s_[:, 128:] = sin[:, 128:].astype(f32) * w_eff[None, :128]
        return np.ascontiguousarray(c), np.ascontiguousarray(s_)

    cos_q_eff, sin_q_eff = rope_tabs(wq_eff)
    cos_k_eff, sin_k_eff = rope_tabs(wk_eff)

    tp = np.arange(128, dtype=np.int64)[:, None]
    sf = np.arange(SCW, dtype=np.int64)[None, :]
    masks = np.empty((len(MASK_RS) * 128, SCW), f32)
    for i, r in enumerate(MASK_RS):
        u = sf - tp + r
        masks[i * 128:(i + 1) * 128] = np.where((u >= 0) & (u < WINDOW),
                                                np.float32(0.0), np.float32(NEG))
    ident = np.eye(128, dtype=f32)
    ones = np.ones((128, 1), f32)

    in_maps = []
    for c in range(N_CORES):
        wq_cols = w_qkv[:, 512 * c:512 * (c + 1)]
        wk_cols = w_qkv[:, H * D + 256 * c:H * D + 256 * (c + 1)]
        wv_cols = w_qkv[:, (H + KV) * D + 256 * c:(H + KV) * D + 256 * (c + 1)]
        w_local = np.ascontiguousarray(
            np.concatenate([wq_cols, wk_cols, wv_cols], axis=1).astype(f32))
        wo_local = np.ascontiguousarray(w_o[512 * c:512 * (c + 1), :].astype(f32))
        in_maps.append({
            "hid_t": hid_t, "w_qkv": w_local, "w_o": wo_local,
            "cos_q": cos_q_eff, "sin_q": sin_q_eff,
            "cos_k": cos_k_eff, "sin_k": sin_k_eff,
            "masks": masks, "identity": ident, "ones": ones,
        })
    return in_maps


def kernel(hidden_states, w_qkv, w_o, q_norm_w, k_norm_w, cos, sin):
    nc = _get_nc()
    in_maps = _host_inputs(np.asarray(hidden_states), np.asarray(w_qkv),
                           np.asarray(w_o), np.asarray(q_norm_w),
                           np.asarray(k_norm_w), np.asarray(cos), np.asarray(sin))
    res = run_bass_kernel_spmd(nc, in_maps, core_ids=list(range(N_CORES)))
    acc = res.results[0]["out_t"].astype(np.float32).copy()
    for c in range(1, N_CORES):
        acc += res.results[c]["out_t"]
    return np.ascontiguousarray(acc.T)
